# revision 61
# baseline (speedup 1.0000x reference)
"""Causal multi-head attention (L=S=2048, B=2, H=16, D=128, fp32) on 8 TRN2
NeuronCores.

Sharding: batch*heads (32 head-batches) split 4-per-core; no cross-core comms.

Per-core device kernel (per head):
  - host supplies QT/KT as [D=128, L] (contraction dim on partitions) and
    V_aug as bf16 [128, 16*132]: per s-chunk a [128, 128+1] block whose extra
    column is ones -- the PV matmul then accumulates the softmax denominator
    into output column 128 for free.
  - scoresT[s,l] tile = KT_chunk.T @ QT_tile (float32r matmul, full rate),
    causal diagonal handled by adding a -1e30 lower-triangle mask,
    exp(scale*x) on ScalarE straight out of PSUM into bf16 SBUF,
    out[l, 0:129] += PT.T @ V_aug accumulated over s-chunks in PSUM,
    then rows scaled by 1/denominator and DMAed out.
  - causal skip: only s-chunks with s <= l are computed at 128x512 tile
    granularity (the l-range per s-chunk starts at the diagonal).
"""

import math

import ml_dtypes
import numpy as np

import concourse.bass as bass
import concourse.mybir as mybir
from concourse import bacc
from concourse.bass_utils import run_bass_kernel_spmd
from concourse.tile import TileContext, add_dep_helper

L = 2048
S = 2048
B = 2
H = 16
D = 128
BH = B * H
NCORES = 8
HPC = BH // NCORES  # heads per core
NLC = L // 128  # l-chunks per head
NSC = S // 128  # s-chunks per head
LT = 512  # l-tile width
NT = L // LT
VST = 132  # stride of one packed V_aug block (128 cols V + 1 ones + 3 pad)
SCALE = 1.0 / math.sqrt(D)
NEG = -1.0e30

F32 = mybir.dt.float32
F32R = mybir.dt.float32r
BF16 = mybir.dt.bfloat16
EXP = mybir.ActivationFunctionType.Exp

_TRACE = False
LAST_RESULT = None

# Design knobs (tuned offline with TimelineSim, verified on HW).
CFG = {
    "mask_post": True,  # mask PT after exp (DVE mul bf16) vs psum add pre-exp
    "out_pair": True,   # out psum as 2x[128,258] banks vs 4x[128,129]
    "sc_bufs": 3,       # score psum [128,1024] buffers (2 banks each)
    "pt_bufs": 4,
    "osb_bufs": 3,
    "act_full": False,  # single ACT over the whole group even when ragged
    "in_chunks": 4,     # split per-head input DMAs into N chunks (startup latency)
    "group_si": 2,      # s-chunks per score-psum group (banks = group_si * sc_bufs)
}


def _build_nc(cfg=None, loop_n=0) -> bass.Bass:
    cfg = {**CFG, **(cfg or {})}
    # Bacc (not bare Bass): its compile() legalizes instructions that end up
    # with more semaphore waits than the ISA slot allows (walrus hard-errors
    # on a Matmult with >1 sync wait).
    nc = bacc.Bacc(None, target_bir_lowering=False)
    # qk: kt and qt interleaved in 512-column chunks --
    # [h, d, (kt[0:512] | qt[0:512] | kt[512:1024] | qt[512:1024] | ...)] --
    # so one DMA per chunk feeds both operands (HWDGE issues serialize at
    # ~0.6us each; fewer issues = faster startup).
    qk_d = nc.declare_dram_parameter("qk", [HPC, D, 2 * L], F32R, isOutput=False)
    va_d = nc.declare_dram_parameter("va", [HPC, 128, NSC * VST], BF16, isOutput=False)
    mask_d = nc.declare_dram_parameter("mask", [128, 128], F32, isOutput=False)
    maskb_d = nc.declare_dram_parameter("maskb", [128, 128], BF16, isOutput=False)
    out_d = nc.declare_dram_parameter("out", [HPC, NLC, 128, D], F32, isOutput=True)

    _lpt = cfg.get("lt", LT) // 128
    n_out_banks = cfg.get("pout_bufs") or (
        _lpt // 2 if cfg["out_pair"] else _lpt
    )

    with TileContext(nc) as tc:
        with (
            tc.tile_pool(name="inputs", bufs=2) as inp_pool,
            tc.tile_pool(name="consts", bufs=1) as const_pool,
            tc.tile_pool(name="ptp", bufs=cfg["pt_bufs"]) as pt_pool,
            tc.tile_pool(name="osbp", bufs=cfg["osb_bufs"]) as osb_pool,
            tc.tile_pool(name="smallp", bufs=8) as small_pool,
            tc.tile_pool(name="pscore", bufs=cfg["sc_bufs"], space="PSUM") as psc_pool,
            tc.tile_pool(name="pacc", bufs=n_out_banks, space="PSUM") as pout_pool,
        ):
            gs = cfg["group_si"]
            sc_banks = gs * cfg.get("lt", LT) // 512
            assert sc_banks * cfg["sc_bufs"] + n_out_banks <= 8
            # Mask load goes on the ACT HWDGE ring so head 0's first input
            # chunk is the first DMA issued on the SP ring (each HWDGE issue
            # serializes ~0.6us; the first matmul gates on that chunk).
            if cfg["mask_post"]:
                maskb_sb = const_pool.tile([128, 128], BF16)
                nc.scalar.dma_start(out=maskb_sb, in_=maskb_d[:, :])
            else:
                mask_sb = const_pool.tile([128, 128], F32)
                nc.scalar.dma_start(out=mask_sb, in_=mask_d[:, :])
            # Dummy 1-element exp at program start: triggers the ~2.7us ACT
            # table load while the first input DMAs are still in flight.
            warm = const_pool.tile([128, 1], F32)
            nc.vector.memset(warm, 0.0)
            nc.scalar.activation(warm, warm, EXP)
            if cfg.get("pe_warm", True):
                # Dummy fp32 matmuls (4 cyc/row -- slow on purpose) keep the
                # PE busy through the first input DMA so the HAM clock-gate
                # is released before the first real matmul.
                warm2 = const_pool.tile([128, 256], F32)
                nc.vector.memset(warm2, 0.0)
                psd = psc_pool.tile([128, 1024], F32, tag="sc")
                for _ in range(4):
                    nc.tensor.matmul(
                        psd[0:1, 0:256],
                        warm2[:, 0:1],
                        warm2,
                        start=True,
                        stop=True,
                    )
            import contextlib

            loop_ctx = (
                tc.For_i(
                    0,
                    loop_n,
                    1,
                    hint_engines=(
                        mybir.EngineType.PE,
                        mybir.EngineType.Activation,
                        mybir.EngineType.DVE,
                        mybir.EngineType.SP,
                    ),
                )
                if loop_n
                else contextlib.nullcontext()
            )
            with loop_ctx:
                _build_body(nc, tc, cfg, qk_d, va_d, mask_d, maskb_d, out_d,
                            inp_pool, const_pool, pt_pool, osb_pool, small_pool,
                            psc_pool, pout_pool,
                            maskb_sb if cfg["mask_post"] else mask_sb)
    nc.compile()
    return nc


def _build_body(nc, tc, cfg, qk_d, va_d, mask_d, maskb_d, out_d,
                inp_pool, const_pool, pt_pool, osb_pool, small_pool,
                psc_pool, pout_pool, mask_tile):
    gs = cfg["group_si"]
    if cfg["mask_post"]:
        maskb_sb = mask_tile
    else:
        mask_sb = mask_tile
    if True:
        if True:
            for h in range(HPC):
                qk_sb = inp_pool.tile([128, 2 * L], F32R, tag="qk")
                va_sb = inp_pool.tile([128, NSC * VST], BF16, tag="va")

                def ktc(a):
                    c = a // 512
                    return c * 1024 + (a - c * 512)

                def qtc(a):
                    c = a // 512
                    return c * 1024 + 512 + (a - c * 512)
                if h == 0:
                    plan = cfg.get("in_plan") or [
                        L // cfg["in_chunks"]
                    ] * cfg["in_chunks"]
                else:
                    n2 = cfg.get("in_chunks_rest", cfg["in_chunks"])
                    plan = [L // n2] * n2
                assert sum(plan) == L
                in_eng = nc.gpsimd if cfg.get("in_swdge") else nc.sync
                pos = 0
                for w in plan:
                    assert pos % 512 == 0 and w % 512 == 0
                    qs = slice(2 * pos, 2 * (pos + w))
                    in_eng.dma_start(out=qk_sb[:, qs], in_=qk_d[h][:, qs])
                    vs = slice((pos // 128) * VST, ((pos + w) // 128) * VST)
                    in_eng.dma_start(out=va_sb[:, vs], in_=va_d[h][:, vs])
                    pos += w
                # Last head runs its l-tiles largest-first so the kernel's
                # final pipeline drain is the shortest tile. (Its inputs were
                # prefetched during the previous head, so the big tile's full
                # K/V needs are already resident.)
                lt = cfg.get("lt", LT)
                lpt = lt // 128  # l-chunks per tile
                nt = L // lt
                t_order = (
                    range(nt - 1, -1, -1) if h == HPC - 1 else range(nt)
                )
                for t in t_order:
                    # last head runs tiles in reverse, so its final tile is t=0
                    is_final_tile = h == HPC - 1 and t == 0
                    lt0 = t * lt
                    if cfg["out_pair"]:
                        outp_t = [
                            pout_pool.tile(
                                [128, 258], F32, tag="outp", name=f"outp_{h}_{t}_{i}"
                            )
                            for i in range(lpt // 2)
                        ]
                        outp = [
                            outp_t[i // 2][:, (i % 2) * 129 : (i % 2) * 129 + 129]
                            for i in range(lpt)
                        ]
                    else:
                        outp = [
                            pout_pool.tile(
                                [128, 129], F32, tag="outp", name=f"outp_{h}_{t}_{i}"
                            )
                            for i in range(lpt)
                        ]
                    osb = osb_pool.tile([128, lt], F32, tag="osb")
                    nsi = lpt * (t + 1)
                    if cfg.get("smart_groups"):
                        # si's with l0 == 0 (all non-diag + first diag) merge
                        # into one ACT per group; the ragged trailing diag
                        # si's (l0 = 128/256/384) need split ACTs anyway, so
                        # pack them into one final group.
                        nz = nsi - (lpt - 1)  # count of l0==0 si's
                        groups = [
                            tuple(range(a, min(a + gs, nz)))
                            for a in range(0, nz, gs)
                        ]
                        if lpt > 1:
                            groups.append(tuple(range(nz, nsi)))
                    else:
                        groups = [
                            tuple(range(g * gs, min((g + 1) * gs, nsi)))
                            for g in range((nsi + gs - 1) // gs)
                        ]
                    for sis in groups:
                        sc = psc_pool.tile([128, len(sis) * lt], F32, tag="sc")
                        pt = pt_pool.tile([128, len(sis) * lt], BF16, tag="pt")
                        offs = []
                        for j, si in enumerate(sis):
                            l0 = max(0, si * 128 - lt0)
                            offs.append(l0)
                            kc = ktc(si * 128)
                            for bs in range(l0 // 512 * 512, lt, 512):
                                mo = max(l0, bs)
                                mw = bs + 512 - mo
                                qc = qtc(lt0 + mo)
                                nc.tensor.matmul(
                                    sc[:, j * lt + mo : j * lt + mo + mw],
                                    qk_sb[:, kc : kc + 128],
                                    qk_sb[:, qc : qc + mw],
                                    start=True,
                                    stop=True,
                                )
                            if si >= lpt * t and not cfg["mask_post"]:
                                dj = j * lt + (si * 128 - lt0)
                                nc.vector.tensor_add(
                                    sc[:, dj : dj + 128],
                                    sc[:, dj : dj + 128],
                                    mask_sb,
                                )
                        if all(o == 0 for o in offs) or cfg["act_full"]:
                            w = len(sis) * lt
                            nc.scalar.activation(
                                pt[:, 0:w], sc[:, 0:w], EXP, scale=SCALE
                            )
                        else:
                            for j, si in enumerate(sis):
                                l0 = offs[j]
                                nc.scalar.activation(
                                    pt[:, j * lt + l0 : (j + 1) * lt],
                                    sc[:, j * lt + l0 : (j + 1) * lt],
                                    EXP,
                                    scale=SCALE,
                                )
                        if cfg["mask_post"]:
                            for j, si in enumerate(sis):
                                if si >= lpt * t:
                                    dj = j * lt + (si * 128 - lt0)
                                    nc.vector.tensor_mul(
                                        pt[:, dj : dj + 128],
                                        pt[:, dj : dj + 128],
                                        maskb_sb,
                                    )
                        for j, si in enumerate(sis):
                            for lc in range(max(si, lpt * t), lpt * (t + 1)):
                                li = lc - lpt * t
                                po = j * lt + (lc * 128 - lt0)
                                # PSUM start=True clears has_written at 2KB
                                # (whole-bank) granularity. With out_pair the
                                # two lc's share a bank: only the even member
                                # starts; the odd member's first write lands
                                # on the bank-wide pending-zero instead (and
                                # must stay ordered after the even start).
                                pair = cfg["out_pair"]
                                mm_start = (si == 0) and (not pair or li % 2 == 0)
                                mm = nc.tensor.matmul(
                                    outp[li],
                                    pt[:, po : po + 128],
                                    va_sb[:, si * VST : si * VST + 129],
                                    start=mm_start,
                                    stop=(si == lc),
                                    skip_group_check=pair,
                                )
                                if pair and si == 0:
                                    if li % 2 == 0:
                                        pair_start_mm = mm
                                    else:
                                        add_dep_helper(
                                            mm.ins,
                                            pair_start_mm.ins,
                                            sync=False,
                                            reason="pair bank start order",
                                        )
                                if si == lc:
                                    rec = small_pool.tile(
                                        [128, 1], F32, tag="rec", name=f"rec_{h}_{lc}"
                                    )
                                    nc.vector.reciprocal(rec, outp[li][:, 128:129])
                                    nc.vector.tensor_scalar_mul(
                                        osb[:, li * 128 : (li + 1) * 128],
                                        outp[li][:, 0:128],
                                        rec,
                                    )
                                    if cfg.get("dma_lc") or (
                                        cfg.get("dma_lc_tail", False)
                                        and is_final_tile
                                    ):
                                        nc.sync.dma_start(
                                            out=out_d[h, lc],
                                            in_=osb[:, li * 128 : (li + 1) * 128],
                                        )
                    if not (
                        cfg.get("dma_lc")
                        or (cfg.get("dma_lc_tail", False) and is_final_tile)
                    ):
                        out_eng = (
                            nc.scalar if cfg.get("out_on_act", False) else nc.sync
                        )
                        out_eng.dma_start(
                            out=out_d[h, lpt * t : lpt * (t + 1)].rearrange(
                                "l p d -> p l d"
                            ),
                            in_=osb,
                        )


_nc_cache = None


def _get_nc():
    global _nc_cache
    if _nc_cache is None:
        _nc_cache = _build_nc()
    return _nc_cache


def _make_in_maps(query, key, value):
    q = np.ascontiguousarray(np.asarray(query, dtype=np.float32).reshape(L, BH, D))
    k = np.ascontiguousarray(np.asarray(key, dtype=np.float32).reshape(S, BH, D))
    v = np.ascontiguousarray(np.asarray(value, dtype=np.float32).reshape(S, BH, D))
    tri = np.arange(128)[:, None] > np.arange(128)[None, :]
    mask = np.where(tri, NEG, 0.0).astype(np.float32)
    maskb = np.where(tri, 0.0, 1.0).astype(ml_dtypes.bfloat16)
    in_maps = []
    for c in range(NCORES):
        hs = slice(c * HPC, (c + 1) * HPC)
        qt = q[:, hs, :].transpose(1, 2, 0)  # [h, d, l]
        kt = k[:, hs, :].transpose(1, 2, 0)  # [h, d, s]
        # interleave kt and qt in 512-col blocks: [kt_c0 | qt_c0 | kt_c1 | ...]
        qk = np.empty((HPC, D, 2 * L), dtype=np.float32)
        qkv = qk.reshape(HPC, D, L // 512, 2, 512)
        qkv[:, :, :, 0, :] = kt.reshape(HPC, D, L // 512, 512)
        qkv[:, :, :, 1, :] = qt.reshape(HPC, D, L // 512, 512)
        vv = v[:, hs, :].transpose(1, 0, 2)  # [h, s, d]
        va = np.zeros((HPC, 128, NSC, VST), dtype=ml_dtypes.bfloat16)
        vt = vv.reshape(HPC, NSC, 128, D).transpose(0, 2, 1, 3)  # [h, p, si, d]
        va[..., :D] = vt.astype(ml_dtypes.bfloat16)
        va[..., D] = 1.0
        in_maps.append(
            {
                "qk": qk,
                "va": np.ascontiguousarray(va.reshape(HPC, 128, NSC * VST)),
                "mask": mask,
                "maskb": maskb,
            }
        )
    return in_maps


def _slope_bench(nc, in_maps, iters=32):
    """Build + warm a jitted single-dispatch runner for `nc`; return
    (best_single_dispatch_ns, run_n) when iters==1, else the async-dispatch
    slope in ns (legacy mode)."""
    import time

    import jax
    from jax.sharding import Mesh, PartitionSpec

    try:
        from jax.experimental.shard_map import shard_map
    except ImportError:
        from jax import shard_map
    from concourse import bass2jax, mybir as _mb

    partition_name = nc.partition_id_tensor.name if nc.partition_id_tensor else None
    in_names, out_names, out_avals, zero_outs = [], [], [], []
    for alloc in nc.m.functions[0].allocations:
        if not isinstance(alloc, mybir.MemoryLocationSet):
            continue
        name = alloc.memorylocations[0].name
        if alloc.kind == "ExternalInput":
            if name != partition_name:
                in_names.append(name)
        elif alloc.kind == "ExternalOutput":
            out_names.append(name)
            shape = tuple(alloc.tensor_shape)
            dtype = _mb.dt.np(alloc.dtype)
            out_avals.append(jax.core.ShapedArray(shape, dtype))
            zero_outs.append(np.zeros(shape, dtype))
    n_params = len(in_names)
    all_names = in_names + out_names
    if partition_name is not None:
        all_names = all_names + [partition_name]

    def _body(*args):
        operands = list(args)
        if partition_name is not None:
            operands.append(bass2jax.partition_id_tensor())
        return tuple(
            bass2jax._bass_exec_p.bind(
                *operands,
                out_avals=tuple(out_avals),
                in_names=tuple(all_names),
                out_names=tuple(out_names),
                lowering_input_output_aliases=(),
                sim_require_finite=True,
                sim_require_nnan=True,
                nc=nc,
            )
        )

    mesh = Mesh(np.asarray(jax.devices()[:NCORES]), ("core",))
    spec_in = (PartitionSpec("core"),) * (n_params + len(out_names))
    spec_out = (PartitionSpec("core"),) * len(out_names)

    concat_in = [
        np.concatenate([np.asarray(in_maps[c][k]) for c in range(NCORES)], axis=0)
        for k in in_names
    ]
    concat_zeros = [
        np.zeros((NCORES * z.shape[0], *z.shape[1:]), z.dtype) for z in zero_outs
    ]
    sharding = jax.sharding.NamedSharding(mesh, PartitionSpec("core"))
    dev_args = [jax.device_put(a, sharding) for a in concat_in + concat_zeros]

    fn = jax.jit(
        shard_map(
            _body, mesh=mesh, in_specs=spec_in, out_specs=spec_out, check_rep=False
        )
    )
    jax.block_until_ready(fn(*dev_args))  # compile + warm

    def run_n(n):
        # async-dispatch n executions back-to-back, block once at the end;
        # the device queue serializes them so the slope is per-exec time.
        t0 = time.perf_counter()
        o = None
        for _ in range(n):
            o = fn(*dev_args)
        jax.block_until_ready(o)
        return time.perf_counter() - t0

    run_n(2)
    if iters == 0:
        return None, run_n
    if iters == 1:
        t1 = min(run_n(1) for _ in range(9))
        return t1 * 1e9, None
    t1 = min(run_n(1) for _ in range(7))
    tn = min(run_n(iters) for _ in range(3))
    per_iter_ns = (tn - t1) / (iters - 1) * 1e9
    return per_iter_ns, {1: t1, iters: tn}


def _bench(query, key, value, ns=(8, 40, 72), rounds=9):
    """Device-side timing: NEFFs whose whole body runs inside a tc.For_i
    loop (one per iteration count in `ns`); dispatches are interleaved
    round-robin so host/axon drift hits all depths equally, then per-depth
    minima are fit with least squares. Cancels the ~80ms axon dispatch
    floor; includes ~2-6us/iter loop back-edge overhead (slightly
    pessimistic vs the single-shot kernel)."""
    import time

    in_maps = _make_in_maps(query, key, value)
    runners = {}
    for n in ns:
        _, run_n = _slope_bench(_build_nc(loop_n=n), in_maps, iters=0)
        runners[n] = run_n
    times = {n: [] for n in ns}
    for _ in range(rounds):
        for n in ns:
            times[n].append(runners[n](1))
    mins = {n: min(v) for n, v in times.items()}
    xs = np.array(list(ns), dtype=np.float64)
    ys = np.array([mins[n] for n in ns], dtype=np.float64)
    slope = ((xs - xs.mean()) * (ys - ys.mean())).sum() / ((xs - xs.mean()) ** 2).sum()
    return slope * 1e9, {n: mins[n] for n in ns}


def _build_floor_nc() -> bass.Bass:
    """Minimal kernel (tiny copy) to calibrate the per-dispatch floor."""
    nc = bacc.Bacc(None, target_bir_lowering=False)
    a = nc.declare_dram_parameter("a", [128, 128], F32, isOutput=False)
    o = nc.declare_dram_parameter("o", [128, 128], F32, isOutput=True)
    with TileContext(nc) as tc:
        with tc.tile_pool(name="sb", bufs=1) as pool:
            ta = pool.tile([128, 128], F32)
            nc.sync.dma_start(out=ta, in_=a[:, :])
            nc.sync.dma_start(out=o[:, :], in_=ta)
    nc.compile()
    return nc


def kernel(query, key, value):
    global LAST_RESULT
    in_maps = _make_in_maps(query, key, value)
    try:
        res = run_bass_kernel_spmd(
            _get_nc(), in_maps, list(range(NCORES)), trace=_TRACE
        )
    except ModuleNotFoundError:
        # trace path needs the axon NTFF hook, absent in some containers
        res = run_bass_kernel_spmd(
            _get_nc(), in_maps, list(range(NCORES)), trace=False
        )
    LAST_RESULT = res
    out = np.empty((L, BH, D), dtype=np.float32)
    for c in range(NCORES):
        o = np.asarray(res.results[c]["out"])  # [HPC, NLC, 128, D]
        out[:, c * HPC : (c + 1) * HPC, :] = o.reshape(HPC, L, D).transpose(1, 0, 2)
    return out.reshape(L, B, H, D)
